# revision 40
# baseline (speedup 1.0000x reference)
"""DifferentialAttention Trainium2 Bass kernel.

Sharding: 8 cores = 2 batches x 4 head-pairs (2 heads each).
Per core (SPMD, same program, different data):
  phase A: q/k/v projections (bf16 matmuls, f32 PSUM) + RMS-norm + PE
           transpose into [d, t] layout. The reference's "rotary" uses the
           head index as its position, giving a constant orthogonal rotation
           per head identical for q and k -- it cancels in q.k^T and is
           skipped entirely. x streamed in 512-column waves so the m-loop
           never starves behind one monolithic tail DMA.
  phase B+C fused, chunk-major: differential causal attention per 512-token
           query chunk (exp without max-subtract; softmax denominator via
           ones-matmuls; per-branch output normalized and freed early so two
           PSUM banks suffice), with the previous chunk's output-projection
           m-tiles issued between S and PV as PE filler for the exp waits,
           spreading the y DMA across the whole phase. y emitted bf16;
           per-core partial y summed over head-pair cores on host.

fp8 DoubleRow was tried and reverted: it bursts at 2x for ~15us, then the
power manager throttles the PE to a 50% utilization limit, capping the
sustained MAC rate at exactly the bf16 rate (throttle_activity_1 17.6%,
util limit 0.5) -- while costing 1.5x the instructions for hi/lo accuracy.
"""

import json
import os
import sys
import tempfile
from contextlib import ExitStack

import numpy as np

sys.path.insert(0, "/opt/trn_rl_repo")

import ml_dtypes  # noqa: E402

import concourse.bass as bass  # noqa: E402
import concourse.mybir as mybir  # noqa: E402
import concourse.tile as tile  # noqa: E402
from concourse import bacc, bass_utils  # noqa: E402
from concourse.masks import make_identity  # noqa: E402

B, T, C = 2, 2048, 2048
NH, HD, HH = 8, 256, 128
LAMBDA_INIT = 0.2
RMS_EPS = 1.1920929e-07
SCALE = float(1.0 / np.sqrt(np.float32(HH)))

F32 = mybir.dt.float32
BF16 = mybir.dt.bfloat16
NPBF16 = ml_dtypes.bfloat16

NM = T // 128          # 16 m-tiles (t blocks)
NK = C // 128          # 16 k-tiles (c blocks)
NCH = T // 512         # 4 tq chunks

_ACT_TABLES_DONE = False


def _setup_act_tables():
    """Reorder act_info so `natural_log_exp_and_others` is the first table:
    it covers every ACT func we use (square, ln, exp, copy), so the greedy
    table selector stays on one table instead of thrashing exp<->ln loads."""
    global _ACT_TABLES_DONE
    if _ACT_TABLES_DONE:
        return
    from neuronxcc.driver.Job import Job  # noqa: PLC0415
    from neuronxcc.driver.jobs.support.FindActInfo import (  # noqa: PLC0415
        findActInfoFile,
    )

    src = findActInfoFile(Job.getPackageDir(), "gen3")
    srcdir = os.path.dirname(src)
    with open(src) as f:
        info = json.load(f)
    info["act_func_sets"].sort(
        key=lambda s: s["name"] != "natural_log_exp_and_others")
    dstdir = os.path.join(tempfile.gettempdir(), "act_info_nlexp_first")
    os.makedirs(dstdir, exist_ok=True)
    for name in os.listdir(srcdir):
        dst = os.path.join(dstdir, name)
        if not os.path.exists(dst):
            try:
                os.symlink(os.path.join(srcdir, name), dst)
            except OSError:
                pass
    act_path = os.path.join(dstdir, "act_info.json")
    with open(act_path, "w") as f:
        json.dump(info, f)
    os.environ["BASS_ACT_ROOT_JSON_PATH"] = act_path

    import concourse.hw_specs as hw_specs  # noqa: PLC0415

    def patched(module_arch):
        return {
            e["name"]: {
                mybir.ActivationFunctionType.from_pwp(v) for v in e["act"]
            }
            for e in info["act_func_sets"]
        }

    hw_specs.get_activation_tables = patched
    bacc.get_activation_tables = patched
    _ACT_TABLES_DONE = True


def _bcast_cols(ap2d, col0, nblk, inner):
    """[128, nblk, inner] view of columns col0..col0+nblk of a [128, n] tile,
    each column replicated `inner` times along a 0-stride inner dim."""
    return bass.AP(
        tensor=ap2d.tensor,
        offset=ap2d.offset + col0,
        ap=[ap2d.ap[0], [1, nblk], [0, inner]],
    )


def _body(tc, aps):
    nc = tc.nc
    xP, wqP, wkP, wvP, woP, tri, ones, neglam, y = aps

    # host-packed partition-major layouts: DMA rows are 4-16KB contiguous
    # per partition (1KB rows run at roughly half the HBM peak)
    xP_r = xP.rearrange("p (c k j) -> p c k j", c=4, k=NK)  # [128,4,16,512]
    wqP_r = wqP.rearrange("p (k j) -> p k j", k=NK)         # [128, 16, 512]
    wkP_r = wkP.rearrange("p (k j) -> p k j", k=NK)
    wvP_r = wvP.rearrange("p (k j) -> p k j", k=NK)
    woP_r = woP.rearrange("p (k j) -> p k j", k=4)          # [128, 4, 2048]

    with ExitStack() as octx:
        # ---- persistent tiles (live across phases) ----
        persist = octx.enter_context(tc.tile_pool(name="persist", bufs=1))
        qkT_all = persist.tile([128, 8, T], BF16)    # seg h*4 + [q1,q2,k1,k2]
        v_all = persist.tile([128, NM, 512], BF16)   # [t(P) per m, e: h0|h1]
        tri_sb = persist.tile([128, 128], BF16)
        ones_sb = persist.tile([128, 1], BF16)
        neglam_sb = persist.tile([128, 1], F32)
        ident_sb = persist.tile([128, 128], BF16)

        nc.sync.dma_start(out=tri_sb, in_=tri)
        nc.sync.dma_start(out=ones_sb, in_=ones)
        nc.sync.dma_start(out=neglam_sb, in_=neglam)
        make_identity(nc, ident_sb)

        # segment mapping: qkT_all viewed [128, h, 4, T]; q -> [:, :, 0:2],
        # k -> [:, :, 2:4]; block order within a group is (h0b1,h0b2,h1b1,h1b2)
        qkT_v = qkT_all.rearrange("p (h f) t -> p h f t", h=2)

        # norm-chain staging outlives phase A: the last two m-tiles' chains
        # and transposes are deferred into the fused phase
        sbA = octx.enter_context(tc.tile_pool(name="sbA", bufs=2))

        # ================= Phase A: projections =================
        with ExitStack() as actx:
            wpool = actx.enter_context(tc.tile_pool(name="wpool", bufs=1))
            xpool = actx.enter_context(tc.tile_pool(name="xpool", bufs=1))
            psA = actx.enter_context(
                tc.tile_pool(name="psA", bufs=6, space="PSUM"))
            psT = actx.enter_context(
                tc.tile_pool(name="psT", bufs=2, space="PSUM"))

            wq_sb = wpool.tile([128, NK, 512], BF16)
            wk_sb = wpool.tile([128, NK, 512], BF16)
            wv_sb = wpool.tile([128, NK, 512], BF16)
            x_sb = xpool.tile([128, NK, T], BF16)
            # DMA order matches the m0-3 prologue's per-projection passes:
            # wq + x wave 1 in 4-kk sub-DMAs (the q-pass trickles behind
            # them), then wk, wv, the x tail in 512-column waves, then wo.
            for g in range(4):
                sl = slice(4 * g, 4 * g + 4)
                nc.sync.dma_start(out=wq_sb[:, sl, :], in_=wqP_r[:, sl, :])
                nc.sync.dma_start(out=x_sb[:, sl, 0:512],
                                  in_=xP_r[:, 0, sl, :])
            for g in range(4):
                sl = slice(4 * g, 4 * g + 4)
                nc.sync.dma_start(out=wk_sb[:, sl, :], in_=wkP_r[:, sl, :])
            for g in range(4):
                sl = slice(4 * g, 4 * g + 4)
                nc.sync.dma_start(out=wv_sb[:, sl, :], in_=wvP_r[:, sl, :])
            for c in range(1, 4):
                nc.sync.dma_start(out=x_sb[:, :, c * 512:(c + 1) * 512],
                                  in_=xP_r[:, c])


            # HAM warmup: identity matmuls fill the DMA-wait head so the PE
            # reaches K=8/8 before the first projection matmul. No DMA deps
            # (identity is gpsimd-generated); the psum scratch slot releases
            # before the prologue needs it.
            wu_ps = psA.tile([128, 128], F32, tag="acc")
            for _ in range(48):
                nc.tensor.matmul(wu_ps, ident_sb, ident_sb,
                                 start=True, stop=True)

            def proj_mm(ps, w_sb, m):
                mlo, mhi = m * 128, (m + 1) * 128
                for kk in range(NK):
                    nc.tensor.matmul(ps, x_sb[:, kk, mlo:mhi],
                                     w_sb[:, kk, :],
                                     start=(kk == 0), stop=(kk == NK - 1))

            def norm_chain(m, qs, ks):
                """RMS stats + normalize; returns the two qnr tiles whose PE
                transposes the caller defers off the m-loop critical path."""
                stats = sbA.tile([128, 8], F32, tag="stats",
                                 name=f"stats{m}")
                sq_scr = sbA.tile([128, 128], F32, tag="sqscr",
                                  name=f"sqscr{m}")
                for j in range(8):
                    src = qs if j < 4 else ks
                    off = (j % 4) * 128
                    nc.scalar.activation(
                        sq_scr, src[:, off:off + 128],
                        mybir.ActivationFunctionType.Square,
                        accum_out=stats[:, j:j + 1])
                # rstd = exp(-0.5*ln(mean+eps)); ln/exp/square share a table
                veps = sbA.tile([128, 8], F32, tag="veps", name=f"veps{m}")
                nc.vector.tensor_scalar(
                    veps, stats, 1.0 / HH, RMS_EPS,
                    mybir.AluOpType.mult, mybir.AluOpType.add)
                lnv = sbA.tile([128, 8], F32, tag="lnv", name=f"lnv{m}")
                nc.scalar.activation(
                    lnv, veps, mybir.ActivationFunctionType.Ln)
                rstd8 = sbA.tile([128, 8], F32, tag="rstd8",
                                 name=f"rstd8{m}")
                nc.scalar.activation(
                    rstd8, lnv, mybir.ActivationFunctionType.Exp, scale=-0.5)

                # normalize (no rotary: the reference's head-axis rotary is a
                # per-head constant orthogonal rotation shared by q and k, so
                # it cancels in q.k^T), batched over the 4 blocks of q then k
                qnrs = []
                for g, (src, c0) in enumerate(((qs, 0), (ks, 4))):
                    qnr = sbA.tile([128, 512], BF16, tag="qnr", bufs=4,
                                   name=f"qnr{m}{g}")
                    nc.vector.tensor_tensor(
                        qnr.rearrange("p (b i) -> p b i", b=4),
                        src.rearrange("p (b i) -> p b i", b=4),
                        _bcast_cols(rstd8, c0, 4, 128), mybir.AluOpType.mult)
                    qnrs.append(qnr)
                return qnrs

            def transposes(m, qnrs, pool=None, tag="tp"):
                mlo, mhi = m * 128, (m + 1) * 128
                for g, qnr in enumerate(qnrs):
                    tp = (pool or psT).tile([128, 512], BF16, tag=tag,
                                            name=f"tp{m}{g}")
                    for bi in range(4):
                        nc.tensor.transpose(
                            tp[:, bi * 128:(bi + 1) * 128],
                            qnr[:, bi * 128:(bi + 1) * 128], ident_sb)
                    # dst: [128, h(2), br(2), 128] at segment group g
                    nc.vector.tensor_copy(
                        qkT_v[:, :, 2 * g:2 * g + 2, mlo:mhi],
                        tp.rearrange("p (h b i) -> p h b i", h=2, b=2))

            # --- prologue m0-3: per-projection passes so the q-pass starts
            # after only wq + x wave 1 (~4MB) instead of the full 8.3MB ---
            # kk-major across the 4 m-tiles: per-kk PE work (4x213ns) slightly
            # exceeds the per-kk DMA (0.26MB), so the PE rides the q-pass
            # trickle with almost no idle.
            def proj_pass(w_sb, tag):
                pss = [psA.tile([128, 512], F32, tag="acc",
                                name=f"p{tag}{m}") for m in range(4)]
                for kk in range(NK):
                    for m in range(4):
                        nc.tensor.matmul(
                            pss[m], x_sb[:, kk, m * 128:(m + 1) * 128],
                            w_sb[:, kk, :],
                            start=(kk == 0), stop=(kk == NK - 1))
                return pss

            pending = []
            q_pss = proj_pass(wq_sb, "q")
            pro_qs = []
            for m in range(4):
                qs = sbA.tile([128, 512], F32, tag="qs", bufs=4,
                              name=f"qs{m}")
                nc.scalar.copy(qs, q_pss[m])
                pro_qs.append(qs)
            k_pss = proj_pass(wk_sb, "k")
            for m in range(4):
                ks = sbA.tile([128, 512], F32, tag="ks", bufs=2,
                              name=f"ks{m}")
                nc.vector.tensor_copy(ks, k_pss[m])
                pending.append((m, norm_chain(m, pro_qs[m], ks)))
            v_pss = proj_pass(wv_sb, "v")
            for m in range(4):
                nc.scalar.copy(v_all[:, m, :], v_pss[m])
                transposes(*pending.pop(0))

            # --- steady state m4+: q/k/v interleaved per kk; transposes of
            # m-1 issued after m's matmuls so the PE never waits on the
            # ACT/DVE norm chain ---
            for m in range(4, NM):
                mlo, mhi = m * 128, (m + 1) * 128
                q_ps = psA.tile([128, 512], F32, tag="acc")
                k_ps = psA.tile([128, 512], F32, tag="acc")
                v_ps = psA.tile([128, 512], F32, tag="acc")
                for kk in range(NK):
                    lhs = x_sb[:, kk, mlo:mhi]
                    st, sp = kk == 0, kk == NK - 1
                    nc.tensor.matmul(q_ps, lhs, wq_sb[:, kk, :], start=st, stop=sp)
                    nc.tensor.matmul(k_ps, lhs, wk_sb[:, kk, :], start=st, stop=sp)
                    nc.tensor.matmul(v_ps, lhs, wv_sb[:, kk, :], start=st, stop=sp)

                if m != NM - 1:
                    while pending:
                        transposes(*pending.pop(0))

                # vacate PSUM quickly: stage q/k to SBUF, v straight out
                qs = sbA.tile([128, 512], F32, tag="qs", bufs=4)
                nc.scalar.copy(qs, q_ps)
                ks = sbA.tile([128, 512], F32, tag="ks", bufs=2)
                nc.vector.tensor_copy(ks, k_ps)
                nc.scalar.copy(v_all[:, m, :], v_ps)
                pending.append((m, norm_chain(m, qs, ks)))
            # m14/m15 transposes stay pending: their ACT/DVE norm chains hide
            # under the fused phase's first S batches

        # ================= Phase B+C fused =================
        with ExitStack() as bctx:
            bcpool = bctx.enter_context(tc.tile_pool(name="bcpool", bufs=1))
            oT_all = bcpool.tile([128, 4, T], BF16)  # seg h*2 + etile
            wo_sb = bcpool.tile([128, 4, T], BF16)
            # wo lands during ch0's attention, well before the first filler
            nc.sync.dma_start(out=wo_sb, in_=woP_r)
            psS = bctx.enter_context(
                tc.tile_pool(name="psS", bufs=4, space="PSUM"))
            psO = bctx.enter_context(
                tc.tile_pool(name="psO", bufs=2, space="PSUM"))
            psY = bctx.enter_context(
                tc.tile_pool(name="psY", bufs=2, space="PSUM"))
            sbPT = bctx.enter_context(tc.tile_pool(name="sbPT", bufs=64))
            sbB = bctx.enter_context(tc.tile_pool(name="sbB", bufs=4))
            sbY = bctx.enter_context(tc.tile_pool(name="sbY", bufs=2))

            # out-projection issued one 512-col cc-group at a time so single
            # groups can be woven into every attention stall point
            ready_cc = []          # FIFO of (m, cc)
            ystages = {}
            budget = [0]           # per-chunk pop allowance

            def pop_filler(n):
                for _ in range(n):
                    if not ready_cc or budget[0] <= 0:
                        return
                    budget[0] -= 1
                    m, cc = ready_cc.pop(0)
                    mlo, mhi = m * 128, (m + 1) * 128
                    if cc == 0:
                        ystages[m] = sbY.tile([128, T], BF16, tag="ystage",
                                              name=f"ystage{m}")
                    ystage = ystages[m]
                    y_ps = psY.tile([128, 512], F32, tag="y",
                                    name=f"y_ps{m}_{cc}")
                    for kk in range(4):
                        nc.tensor.matmul(
                            y_ps, oT_all[:, kk, mlo:mhi],
                            wo_sb[:, kk, cc * 512:(cc + 1) * 512],
                            start=(kk == 0), stop=(kk == 3))
                    if cc % 2 == 0:
                        nc.vector.tensor_copy(
                            ystage[:, cc * 512:(cc + 1) * 512], y_ps)
                    else:
                        nc.scalar.copy(
                            ystage[:, cc * 512:(cc + 1) * 512], y_ps)
                    if cc == 3:
                        nc.sync.dma_start(out=y[mlo:mhi, :], in_=ystage)
                        del ystages[m]

            def stage1(h, ch, slot):
                """S + exp + dens + reciprocals + gpsimd broadcasts for slot
                (h, ch). The broadcasts get a full slot of latency cover
                before stage2 consumes them."""
                c0, c1 = ch * 512, ch * 512 + 512
                ntk = 4 * ch + 4
                qT = [qkT_v[:, h, 0, :], qkT_v[:, h, 1, :]]
                kT = [qkT_v[:, h, 2, :], qkT_v[:, h, 3, :]]
                br_pts = []
                for br in range(2):
                    pts = []
                    for tkb in range(ntk):
                        n0 = max(c0, tkb * 128)
                        nN = c1 - n0
                        col0 = n0 - c0
                        s_ps = psS.tile([128, 512], F32, tag="s")
                        nc.tensor.matmul(
                            s_ps[:, :nN],
                            kT[br][:, tkb * 128:(tkb + 1) * 128],
                            qT[br][:, n0:c1], start=True, stop=True)
                        pt = sbPT.tile([128, 512], BF16, tag="pt")
                        nc.scalar.activation(
                            pt[:, :nN], s_ps[:, :nN],
                            mybir.ActivationFunctionType.Exp, scale=SCALE)
                        if tkb * 128 >= c0:   # diagonal block
                            nc.vector.tensor_mul(
                                pt[:, :128], pt[:, :128], tri_sb)
                        pts.append((tkb, pt, nN, col0))
                    br_pts.append(pts)
                    pop_filler(1 + ntk // 4)
                ibs = []
                for br in range(2):
                    full = [p for p in br_pts[br] if p[2] == 512]
                    rest = [p for p in br_pts[br] if p[2] < 512]
                    items = []
                    for j in range(0, len(full) - 1, 2):
                        pp = sbB.tile([128, 512], BF16, tag="pp",
                                      bufs=8, name=f"pp{slot}{br}{j}")
                        nc.vector.tensor_add(pp, full[j][1], full[j + 1][1])
                        items.append((pp, 512, 0))
                    if len(full) % 2:
                        items.append((full[-1][1], 512, 0))
                    items += [(pt[:, :nN], nN, col0)
                              for (tkb, pt, nN, col0) in rest]
                    dp = psS.tile([1, 512], F32, tag="s",
                                  name=f"den{slot}{br}")
                    for i, (rhs, nN, col0) in enumerate(items):
                        nc.tensor.matmul(
                            dp[:, col0:], ones_sb, rhs[:, :nN],
                            start=(i == 0), stop=(i == len(items) - 1))
                    inv = sbB.tile([1, 512], F32, tag="inv",
                                   name=f"inv{slot}{br}")
                    nc.vector.reciprocal_approx_fast(inv, dp)
                    ib = sbB.tile([128, 512], F32, tag="invb", bufs=6,
                                  name=f"invb{slot}{br}")
                    nc.gpsimd.partition_broadcast(ib, inv)
                    ibs.append(ib)
                return (h, ch, ntk, br_pts, ibs)

            def stage2(ctx):
                """PV + normalize + combine for a slot prepared by stage1."""
                h, ch, ntk, br_pts, ibs = ctx
                c0, c1 = ch * 512, ch * 512 + 512
                onorm = [[None, None], [None, None]]
                for br in range(2):
                    for e in range(2):
                        ecol = h * 256 + e * 128
                        o_ps = psO.tile([128, 512], F32, tag="o",
                                        name=f"o_ps{br}{e}")
                        for i, (tkb, pt, nN, col0) in enumerate(br_pts[br]):
                            nc.tensor.matmul(
                                o_ps[:, col0:],
                                v_all[:, tkb, ecol:ecol + 128],
                                pt[:, :nN],
                                start=(i == 0), stop=(i == ntk - 1))
                        on = sbB.tile([128, 512], F32, tag=f"on{br}", bufs=2,
                                      name=f"on{h}{ch}{br}{e}")
                        nc.vector.tensor_mul(on, o_ps, ibs[br])
                        onorm[br][e] = on
                        pop_filler(1)
                for e in range(2):
                    nc.vector.scalar_tensor_tensor(
                        oT_all[:, h * 2 + e, c0:c1], onorm[1][e],
                        neglam_sb, onorm[0][e],
                        mybir.AluOpType.mult, mybir.AluOpType.add)
                if h == 1:
                    ready_cc.extend((4 * ch + i, cc) for i in range(4)
                                    for cc in range(4))

            # software pipeline, two slots deep through the small head
            # chunks (covers the gpsimd broadcast's multi-us semaphore-poll
            # latency), one slot deep after; pop budgets hold back part of
            # the out-projection stream so the exp-bound ch3 keeps PE filler
            ctxs = {}

            def s1(h, ch):
                ctxs[(h, ch)] = stage1(h, ch, f"{h}{ch}")

            def s2(h, ch):
                stage2(ctxs.pop((h, ch)))

            s1(0, 0)
            s1(1, 0)
            # deferred m14/m15 transposes: their norm chains ran during the
            # S batches above; psS has spare bank room here
            while pending:
                transposes(*pending.pop(0), psS, "s")
            s1(0, 1)
            s2(0, 0)
            s2(1, 0)
            budget[0] = 10
            s1(1, 1)
            s2(0, 1)
            s1(0, 2)
            s2(1, 1)
            budget[0] = 10
            s1(1, 2)
            s2(0, 2)
            s1(0, 3)
            s2(1, 2)
            budget[0] = 999
            s1(1, 3)
            s2(0, 3)
            s2(1, 3)
            pop_filler(len(ready_cc))


def build_nc():
    try:
        _setup_act_tables()
    except Exception:
        pass  # fall back to default tables (correct, extra table loads)
    nc = bacc.Bacc("TRN2", target_bir_lowering=False, debug=False,
                   num_devices=8)
    xP = nc.dram_tensor("xP", [128, 4 * NK * 512], BF16,
                        kind="ExternalInput").ap()
    wqP = nc.dram_tensor("wqP", [128, NK * 512], BF16,
                         kind="ExternalInput").ap()
    wkP = nc.dram_tensor("wkP", [128, NK * 512], BF16,
                         kind="ExternalInput").ap()
    wvP = nc.dram_tensor("wvP", [128, NK * 512], BF16,
                         kind="ExternalInput").ap()
    woP = nc.dram_tensor("woP", [128, 4 * T], BF16,
                         kind="ExternalInput").ap()
    tri = nc.dram_tensor("tri", [128, 128], BF16, kind="ExternalInput").ap()
    ones = nc.dram_tensor("ones", [128, 1], BF16, kind="ExternalInput").ap()
    neglam = nc.dram_tensor("neglam", [128, 1], F32,
                            kind="ExternalInput").ap()
    y = nc.dram_tensor("y", [T, C], BF16, kind="ExternalOutput").ap()
    with tile.TileContext(nc) as tc:
        _body(tc, (xP, wqP, wkP, wvP, woP, tri, ones, neglam, y))
    nc.compile()
    return nc


def _host_prep(x, wq, wk, wv, wo, lq1, lk1, lq2, lk2):
    x = np.asarray(x, np.float32)
    wq, wk, wv, wo = (np.asarray(w, np.float32) for w in (wq, wk, wv, wo))
    lam = float(np.exp(np.sum(np.asarray(lq1, np.float32) *
                              np.asarray(lk1, np.float32))) -
                np.exp(np.sum(np.asarray(lq2, np.float32) *
                              np.asarray(lk2, np.float32))) + LAMBDA_INIT)

    tri = np.triu(np.ones((128, 128), np.float32)).astype(NPBF16)
    ones = np.ones((128, 1), np.float32).astype(NPBF16)
    neglam = np.full((128, 1), -lam, np.float32)

    in_maps = []
    for core in range(8):
        b = core // 4
        hp = core % 4
        h0, h1 = 2 * hp, 2 * hp + 1
        rows = np.r_[h0 * 256:(h0 + 1) * 256, h1 * 256:(h1 + 1) * 256]
        # partition-major packs matching the kernel's SBUF layouts, so each
        # DMA row is 4-16KB contiguous per partition
        xT = x[b].T                                    # [C, T]
        xPk = xT.reshape(NK, 128, 4, 512).transpose(1, 2, 0, 3)
        wqPk = wq[rows, :].T.reshape(NK, 128, 512).transpose(1, 0, 2)
        wkPk = wk[rows, :].T.reshape(NK, 128, 512).transpose(1, 0, 2)
        wvPk = wv[rows, :].T.reshape(NK, 128, 512).transpose(1, 0, 2)
        woPk = (wo[:, rows].T * (1.0 - LAMBDA_INIT)).reshape(
            4, 128, T).transpose(1, 0, 2)
        in_maps.append({
            "xP": np.ascontiguousarray(xPk.reshape(128, -1)).astype(NPBF16),
            "wqP": np.ascontiguousarray(wqPk.reshape(128, -1)).astype(NPBF16),
            "wkP": np.ascontiguousarray(wkPk.reshape(128, -1)).astype(NPBF16),
            "wvP": np.ascontiguousarray(wvPk.reshape(128, -1)).astype(NPBF16),
            "woP": np.ascontiguousarray(woPk.reshape(128, -1)).astype(NPBF16),
            "tri": tri,
            "ones": ones,
            "neglam": neglam,
        })
    return in_maps


def kernel(x, wq, wk, wv, wo, lq1, lk1, lq2, lk2, _results_out=None,
           _trace=False):
    in_maps = _host_prep(x, wq, wk, wv, wo, lq1, lk1, lq2, lk2)
    nc = build_nc()
    res = bass_utils.run_bass_kernel_spmd(nc, in_maps,
                                          core_ids=list(range(8)),
                                          trace=_trace)
    if _results_out is not None:
        _results_out.append(res)
    out = np.zeros((B, T, C), np.float32)
    for core in range(8):
        out[core // 4] += res.results[core]["y"].astype(np.float32)
    return out
